# revision 4
# baseline (speedup 1.0000x reference)
"""GNN segment-softmax attention aggregation on 8 TRN2 NeuronCores.

Math (reference): q = x_j + e_ij; src = tanh([q, x_i] @ W + b)  [E,1]
  w = segment_softmax(src, index); out = segment_sum(w * msg)   [N,32]

tanh bounds src to (-1,1) so exp never overflows -> drop the (detached)
segment-max: out_n = T_n / (S_n + 1e-16), T_n = sum exp(src)*msg,
S_n = sum exp(src).

Device mapping (v2, engine-balanced):
  * Host (untimed) pads/permutes edges into G=8 slots per node-group; one
    group per SBUF partition (as baseline).
  * Score dot-products on the TensorEngine: per (tile, slot) a [96,128]
    feature-major stationary (xj|eij|xi rows) x Wcat [96,1] -> psum column
    of dots for 128 groups.  64 MM per super-tile into one [128,64] psum.
  * tanh+exp batched on ScalarE (one op per super each).
  * DVE does only 2x-mode work: mask mult, msg*u (broadcast-mid view),
    add-trees over G (instead of 1x tensor_reduce), one-hot is_equal.
  * Per-tile segment-reduce via one-hot matmul into [128,33] psum
    (as baseline), copy on ScalarE, DMA out.
  * Edge-parallel across 8 cores, no collectives; host combines the tiny
    per-tile node partials and divides.
"""

import os
import sys

import numpy as np
from ml_dtypes import bfloat16 as np_bf16

for _p in ("/opt/trn_rl_repo", "/root/.axon_site/_ro/trn_rl_repo"):
    if os.path.isdir(_p) and _p not in sys.path:
        sys.path.insert(0, _p)

from concourse import bacc, bass, mybir, tile  # noqa: E402
from concourse.bass_utils import run_bass_kernel_spmd  # noqa: E402


def _ensure_ntff_hook():
    """This image's antenv lacks axon_hooks; recreate it so trace=True
    (BASS_TRACE=1) can capture NTFF exec_time_ns via libaxon_pjrt."""
    import types

    if "antenv.axon_hooks" in sys.modules:
        return
    try:
        mod = types.ModuleType("antenv.axon_hooks")
        state = {"h": None}
        mod.set_axon_ntff_profile_hook = lambda h: state.__setitem__("h", h)
        mod.get_axon_ntff_profile_hook = lambda: state["h"]
        sys.modules["antenv.axon_hooks"] = mod
        import antenv

        antenv.axon_hooks = mod
        from trn_agent_boot.trn_boot import _ntff_profile_via_ctypes

        so = "/opt/axon/libaxon_pjrt.so"
        if os.path.exists(so):
            mod.set_axon_ntff_profile_hook(_ntff_profile_via_ctypes(so))
    except Exception:
        pass


_ensure_ntff_hook()

G = 8          # edge slots per group (one group = one node's slots, one SBUF partition)
D = 32         # feature dim
NCORES = 8
S = 8          # tiles per super-tile
LAST_EXEC_NS = None

_PROGRAM_CACHE = {}


def _build_program(ntiles: int, bval: float):
    f32 = mybir.dt.float32
    bf16 = mybir.dt.bfloat16
    nc = bacc.Bacc(None, target_bir_lowering=False, debug=False)

    nsup = ntiles // S
    SG = S * G                      # 64 slot-columns per super
    # score pack: per super [96, S*G*128] feature-major (xj|eij|xi rows)
    sc_d = nc.declare_dram_parameter("sc", [nsup, 96, SG * 128], bf16, isOutput=False)
    # msg pack: per super [128, S, D, G] (transposed per group so u broadcasts
    # over the middle D dim with unit inner stride)
    mg_d = nc.declare_dram_parameter("mg", [nsup, 128, S * D * G], bf16, isOutput=False)
    msk_d = nc.declare_dram_parameter("mask", [128, ntiles, G], bf16, isOutput=False)
    rel_d = nc.declare_dram_parameter("rel", [128, ntiles], f32, isOutput=False)
    wc_d = nc.declare_dram_parameter("wcat", [96, 1], bf16, isOutput=False)
    out_d = nc.declare_dram_parameter(
        "out", [nsup, 128, S * (D + 1)], f32, isOutput=True
    )

    ALU = mybir.AluOpType
    ACT = mybir.ActivationFunctionType

    with tile.TileContext(nc) as tc:
        with (
            tc.tile_pool(name="const", bufs=1) as constp,
            tc.tile_pool(name="scp", bufs=2) as scp,
            tc.tile_pool(name="mgp", bufs=2) as mgp,
            tc.tile_pool(name="work", bufs=2) as workp,
            tc.tile_pool(name="small", bufs=2) as smallp,
            tc.tile_pool(name="ohp", bufs=4) as ohp,
            tc.tile_pool(name="obp", bufs=2) as obp,
            tc.tile_pool(name="psc", bufs=2, space="PSUM") as pscp,
            tc.tile_pool(name="pst", bufs=4, space="PSUM") as pstp,
        ):
            wcat = constp.tile([96, 1], bf16)
            nc.sync.dma_start(out=wcat[:], in_=wc_d[:])
            maskall = constp.tile([128, ntiles, G], bf16)
            nc.sync.dma_start(out=maskall[:], in_=msk_d[:])
            relall = constp.tile([128, ntiles], f32)
            nc.sync.dma_start(out=relall[:], in_=rel_d[:])
            iota_t = constp.tile([128, 128], bf16)
            nc.gpsimd.iota(
                iota_t[:],
                pattern=[[1, 128]],
                base=0,
                channel_multiplier=0,
                allow_small_or_imprecise_dtypes=True,
            )

            for sp in range(nsup):
                sc = scp.tile([96, SG * 128], bf16, tag="sc")
                nc.sync.dma_start(out=sc[:], in_=sc_d[sp])
                mg = mgp.tile([128, S, D, G], bf16, tag="mg")
                nc.sync.dma_start(
                    out=mg[:].rearrange("p s d g -> p (s d g)"), in_=mg_d[sp]
                )

                # --- scores on PE: per slot-column 4 col-tiled [96,32] MMs ---
                # (concurrent on disjoint 32-col strips of the PE array)
                dots_ps = pscp.tile([128, SG], f32)
                for c in range(SG):
                    for j in range(4):
                        nc.tensor.matmul(
                            dots_ps[32 * j : 32 * (j + 1), c : c + 1],
                            sc[:, c * 128 + 32 * j : c * 128 + 32 * (j + 1)],
                            wcat[:],
                            start=True,
                            stop=True,
                            tile_position=(0, 32 * j),
                        )

                # --- u = exp(tanh(dots + b)) on ScalarE, then mask on DVE ---
                th = smallp.tile([128, SG], f32, tag="th")
                nc.scalar.activation(th[:], dots_ps[:], ACT.Tanh, bias=bval)
                u0 = smallp.tile([128, SG], bf16, tag="u0")
                nc.scalar.activation(u0[:], th[:], ACT.Exp)
                um = smallp.tile([128, S, G], bf16, tag="um")
                nc.vector.tensor_tensor(
                    um[:].rearrange("p s g -> p (s g)"),
                    u0[:],
                    maskall[:, sp * S : (sp + 1) * S, :].rearrange(
                        "p s g -> p (s g)"
                    ),
                    op=ALU.mult,
                )

                # --- weighted msg + trees over g (all 2x tt ops) ---
                rhs = smallp.tile([128, S, D + 1], bf16, tag="rhs")
                wm = workp.tile([128, S, D, G], bf16, tag="wm")
                umb = (
                    um[:]
                    .rearrange("p s (o g) -> p s o g", o=1)
                    .broadcast_to([128, S, D, G])
                )
                nc.vector.tensor_tensor(wm[:], mg[:], umb, op=ALU.mult)
                w4 = workp.tile([128, S, D, 4], bf16, tag="w4")
                nc.vector.tensor_tensor(
                    w4[:], wm[:, :, :, 0:4], wm[:, :, :, 4:8], op=ALU.add
                )
                w2 = workp.tile([128, S, D, 2], bf16, tag="w2")
                nc.vector.tensor_tensor(
                    w2[:], w4[:, :, :, 0:2], w4[:, :, :, 2:4], op=ALU.add
                )
                nc.vector.tensor_tensor(
                    rhs[:, :, 0:D].rearrange("p s (d o) -> p s d o", o=1),
                    w2[:, :, :, 0:1],
                    w2[:, :, :, 1:2],
                    op=ALU.add,
                )
                # S_n tree over g
                s4 = smallp.tile([128, S, 4], bf16, tag="s4")
                nc.vector.tensor_tensor(
                    s4[:], um[:, :, 0:4], um[:, :, 4:8], op=ALU.add
                )
                s2 = smallp.tile([128, S, 2], bf16, tag="s2")
                nc.vector.tensor_tensor(
                    s2[:], s4[:, :, 0:2], s4[:, :, 2:4], op=ALU.add
                )
                nc.vector.tensor_tensor(
                    rhs[:, :, D : D + 1],
                    s2[:, :, 0:1],
                    s2[:, :, 1:2],
                    op=ALU.add,
                )

                # --- per-tile one-hot segment reduce on PE ---
                ob = obp.tile([128, S, D + 1], f32, tag="ob")
                for k in range(S):
                    t = sp * S + k
                    oh = ohp.tile([128, 128], bf16, tag="oh")
                    nc.vector.tensor_scalar(
                        oh[:], iota_t[:], relall[:, t : t + 1], None,
                        op0=ALU.is_equal,
                    )
                    ps = pstp.tile([128, D + 1], f32)
                    nc.tensor.matmul(ps[:], oh[:], rhs[:, k, :], start=True, stop=True)
                    nc.scalar.copy(ob[:, k, :], ps[:])
                nc.sync.dma_start(out=out_d[sp], in_=ob[:])

    nc.compile()
    return nc


def kernel(msg, x_i, x_j, e_ij, W, b, index, num_nodes):
    global LAST_EXEC_NS
    msg = np.ascontiguousarray(np.asarray(msg, dtype=np.float32))
    x_i = np.ascontiguousarray(np.asarray(x_i, dtype=np.float32))
    x_j = np.ascontiguousarray(np.asarray(x_j, dtype=np.float32))
    e_ij = np.ascontiguousarray(np.asarray(e_ij, dtype=np.float32))
    W = np.asarray(W, dtype=np.float32)
    bval = float(np.asarray(b, dtype=np.float32).reshape(-1)[0])
    idx = np.asarray(index).astype(np.int64).reshape(-1)
    N = int(np.asarray(num_nodes).reshape(()))
    E = idx.shape[0]

    # ---- host prep (untimed): pad edges into G-slot groups per node ----
    if np.any(np.diff(idx) < 0):
        order = np.argsort(idx, kind="stable")
    else:
        order = np.arange(E, dtype=np.int64)
    idx_s = idx[order]

    deg = np.bincount(idx_s, minlength=N)
    ngrp = -(-deg // G)
    B = int(ngrp.sum())
    bc = -(-B // NCORES)
    bc = -(-bc // 1024) * 1024  # per-core groups, multiple of 128*S (super-tiles)
    btot = bc * NCORES
    ntiles = bc // 128

    node_of_group = np.repeat(np.arange(N, dtype=np.int64), ngrp)
    node_of_group = np.concatenate(
        [node_of_group, np.full(btot - B, N, dtype=np.int64)]
    )

    gstart = np.zeros(N + 1, dtype=np.int64)
    np.cumsum(ngrp, out=gstart[1:])
    seg_start = np.zeros(N + 1, dtype=np.int64)
    np.cumsum(deg, out=seg_start[1:])
    rank_in_node = np.arange(E, dtype=np.int64) - seg_start[idx_s]
    slot = gstart[idx_s] * G + rank_in_node  # slot of each sorted edge

    nslots = btot * G
    perm = np.full(nslots, -1, dtype=np.int64)
    perm[slot] = order
    mask_f = (perm >= 0).astype(np.float32)
    src_idx = np.where(perm >= 0, perm, 0)

    nsup = ntiles // S
    SG = S * G

    # --- score pack: [C, nsup, 96, S, G, 128] feature-major bf16 ---
    # slot-column c = t*G*128 + g*128 + p ; rows = xj(0:32)|eij(32:64)|xi(64:96)
    # per-core slot layout mirror: slot id = ((tile*128)+p)*G + g
    sc = np.empty((NCORES, nsup, 96, SG * 128), dtype=np_bf16)
    # build index array mapping (tile,p,g) -> src edge, then transpose views
    si = src_idx.reshape(NCORES, nsup, S, 128, G)
    for arr, row0 in ((x_j, 0), (e_ij, 32), (x_i, 64)):
        # arr[si] -> [C, nsup, S, 128, G, 32] ; want [C, nsup, 32, S, G, 128]
        v = arr[si].astype(np_bf16).transpose(0, 1, 5, 2, 4, 3)
        sc[:, :, row0 : row0 + 32] = v.reshape(NCORES, nsup, 32, SG * 128)

    # --- msg pack: [C, nsup, 128, S, D, G] bf16, pad slots zeroed ---
    mgv = (msg[src_idx] * mask_f[:, None]).astype(np_bf16)
    mg = np.ascontiguousarray(
        mgv.reshape(NCORES, nsup, S, 128, G, D).transpose(0, 1, 3, 2, 5, 4)
    ).reshape(NCORES, nsup, 128, S * D * G)

    mk = mask_f.astype(np_bf16).reshape(NCORES, ntiles, 128, G)
    mks = [np.ascontiguousarray(mk[c].transpose(1, 0, 2)) for c in range(NCORES)]

    # per-tile dense rank of node within tile (always < 128), plus row->node map
    nog = node_of_group.reshape(NCORES, ntiles, 128)
    newseg = np.ones((NCORES, ntiles, 128), dtype=np.int64)
    newseg[:, :, 1:] = (np.diff(nog, axis=2) != 0).astype(np.int64)
    rank = np.cumsum(newseg, axis=2) - 1  # [C, T, 128] in [0, 128)
    rels = [
        np.ascontiguousarray(rank[c].T.astype(np.float32)) for c in range(NCORES)
    ]
    nodemap = np.full((NCORES, ntiles, 128), N, dtype=np.int64)
    ci, ti, _ = np.meshgrid(
        np.arange(NCORES), np.arange(ntiles), np.arange(128), indexing="ij"
    )
    nodemap[ci, ti, rank] = nog

    # rows: xj*W1 + eij*W1 + xi*W2  -> [W1, W1, W2]
    wcat = np.concatenate([W[:D, 0], W[:D, 0], W[D:, 0]])
    wcat = np.ascontiguousarray(wcat.reshape(96, 1)).astype(np_bf16)

    in_maps = [
        {
            "sc": np.ascontiguousarray(sc[c]),
            "mg": np.ascontiguousarray(mg[c]),
            "mask": mks[c],
            "rel": rels[c],
            "wcat": wcat,
        }
        for c in range(NCORES)
    ]

    key = (ntiles, bval)
    if key not in _PROGRAM_CACHE:
        _PROGRAM_CACHE[key] = _build_program(ntiles, bval)
    nc = _PROGRAM_CACHE[key]

    res = run_bass_kernel_spmd(nc, in_maps, core_ids=list(range(NCORES)))
    LAST_EXEC_NS = res.exec_time_ns

    acc = np.zeros((N + 1, D + 1), dtype=np.float32)
    for c in range(NCORES):
        o = (
            np.asarray(res.results[c]["out"], dtype=np.float32)
            .reshape(nsup, 128, S, D + 1)
            .transpose(0, 2, 1, 3)
            .reshape(-1, D + 1)
        )
        np.add.at(acc, nodemap[c].reshape(-1), o)
    out = acc[:N, :D] / (acc[:N, D : D + 1] + 1e-16)
    return out.astype(np.float32)


# revision 6
# speedup vs baseline: 2.2045x; 2.2045x over previous
"""GNN segment-softmax attention aggregation on 8 TRN2 NeuronCores.

Math (reference): q = x_j + e_ij; src = tanh([q, x_i] @ W + b)  [E,1]
  w = segment_softmax(src, index); out = segment_sum(w * msg)   [N,32]

tanh bounds src to (-1,1) so exp never overflows -> drop the (detached)
segment-max: out_n = T_n / (S_n + 1e-16), T_n = sum exp(src)*msg,
S_n = sum exp(src).

Device mapping (v2, engine-balanced):
  * Host (untimed) pads/permutes edges into G=8 slots per node-group; one
    group per SBUF partition (as baseline).
  * Score dot-products on the TensorEngine: per (tile, slot) a [96,128]
    feature-major stationary (xj|eij|xi rows) x Wcat [96,1] -> psum column
    of dots for 128 groups.  64 MM per super-tile into one [128,64] psum.
  * tanh+exp batched on ScalarE (one op per super each).
  * DVE does only 2x-mode work: mask mult, msg*u (broadcast-mid view),
    add-trees over G (instead of 1x tensor_reduce), one-hot is_equal.
  * Per-tile segment-reduce via one-hot matmul into [128,33] psum
    (as baseline), copy on ScalarE, DMA out.
  * Edge-parallel across 8 cores, no collectives; host combines the tiny
    per-tile node partials and divides.
"""

import os
import sys

import numpy as np
from ml_dtypes import bfloat16 as np_bf16
from ml_dtypes import float8_e4m3fn as np_fp8

for _p in ("/opt/trn_rl_repo", "/root/.axon_site/_ro/trn_rl_repo"):
    if os.path.isdir(_p) and _p not in sys.path:
        sys.path.insert(0, _p)

from concourse import bacc, bass, mybir, tile  # noqa: E402
from concourse.bass_utils import run_bass_kernel_spmd  # noqa: E402


def _ensure_ntff_hook():
    """This image's antenv lacks axon_hooks; recreate it so trace=True
    (BASS_TRACE=1) can capture NTFF exec_time_ns via libaxon_pjrt."""
    import types

    if "antenv.axon_hooks" in sys.modules:
        return
    try:
        mod = types.ModuleType("antenv.axon_hooks")
        state = {"h": None}
        mod.set_axon_ntff_profile_hook = lambda h: state.__setitem__("h", h)
        mod.get_axon_ntff_profile_hook = lambda: state["h"]
        sys.modules["antenv.axon_hooks"] = mod
        import antenv

        antenv.axon_hooks = mod
        from trn_agent_boot.trn_boot import _ntff_profile_via_ctypes

        so = "/opt/axon/libaxon_pjrt.so"
        if os.path.exists(so):
            mod.set_axon_ntff_profile_hook(_ntff_profile_via_ctypes(so))
    except Exception:
        pass


_ensure_ntff_hook()

G = 8          # edge slots per group (one group = one node's slots, one SBUF partition)
D = 32         # feature dim
NCORES = 8
S = 8          # tiles per super-tile
LAST_EXEC_NS = None

_PROGRAM_CACHE = {}


def _build_program(ntiles: int, bval: float):
    f32 = mybir.dt.float32
    bf16 = mybir.dt.bfloat16
    nc = bacc.Bacc(None, target_bir_lowering=False, debug=False)

    nsup = ntiles // S
    SG = S * G                      # 64 slot-columns per super
    # score pack: per super [96, S*G*128] feature-major (xj|eij|xi rows)
    fp8 = mybir.dt.float8e4
    sc_d = nc.declare_dram_parameter("sc", [nsup, 96, SG * 128], fp8, isOutput=False)
    # msg pack: per super [128, S, D, G] (transposed per group so u broadcasts
    # over the middle D dim with unit inner stride)
    mg_d = nc.declare_dram_parameter("mg", [nsup, 128, S * D * G], bf16, isOutput=False)
    msk_d = nc.declare_dram_parameter("mask", [128, ntiles, G], bf16, isOutput=False)
    rel_d = nc.declare_dram_parameter("rel", [128, ntiles], f32, isOutput=False)
    wc_d = nc.declare_dram_parameter("wcat", [96, 1], bf16, isOutput=False)
    out_d = nc.declare_dram_parameter(
        "out", [nsup, 128, S * (D + 1)], f32, isOutput=True
    )

    ALU = mybir.AluOpType
    ACT = mybir.ActivationFunctionType

    with tile.TileContext(nc) as tc:
        with (
            tc.tile_pool(name="const", bufs=1) as constp,
            tc.tile_pool(name="scp", bufs=2) as scp,
            tc.tile_pool(name="mgp", bufs=2) as mgp,
            tc.tile_pool(name="work", bufs=2) as workp,
            tc.tile_pool(name="small", bufs=2) as smallp,
            tc.tile_pool(name="ohp", bufs=4) as ohp,
            tc.tile_pool(name="obp", bufs=2) as obp,
            tc.tile_pool(name="psc", bufs=2, space="PSUM") as pscp,
            tc.tile_pool(name="pst", bufs=4, space="PSUM") as pstp,
        ):
            wcat = constp.tile([96, 1], bf16)
            nc.sync.dma_start(out=wcat[:], in_=wc_d[:])
            maskall = constp.tile([128, ntiles, G], bf16)
            nc.sync.dma_start(out=maskall[:], in_=msk_d[:])
            relall = constp.tile([128, ntiles], f32)
            nc.sync.dma_start(out=relall[:], in_=rel_d[:])
            iota_t = constp.tile([128, 128], bf16)
            nc.gpsimd.iota(
                iota_t[:],
                pattern=[[1, 128]],
                base=0,
                channel_multiplier=0,
                allow_small_or_imprecise_dtypes=True,
            )

            for sp in range(nsup):
                sc = scp.tile([96, SG * 128], fp8, tag="sc")
                nc.sync.dma_start(out=sc[:], in_=sc_d[sp])
                mg = mgp.tile([128, S, D, G], bf16, tag="mg")
                nc.sync.dma_start(
                    out=mg[:].rearrange("p s d g -> p (s d g)"), in_=mg_d[sp]
                )

                # --- scores on PE: 64 x ([96,128] stationary  @ [96,1]) ---
                dots_ps = pscp.tile([128, SG], f32)
                for c in range(SG):
                    nc.tensor.matmul(
                        dots_ps[:, c : c + 1],
                        sc[:, c * 128 : (c + 1) * 128],
                        wcat[:],
                        start=True,
                        stop=True,
                    )

                # --- u = exp(tanh(dots + b)) on ScalarE, then mask on DVE ---
                th = smallp.tile([128, SG], f32, tag="th")
                nc.scalar.activation(th[:], dots_ps[:], ACT.Tanh, bias=bval)
                u0 = smallp.tile([128, SG], bf16, tag="u0")
                nc.scalar.activation(u0[:], th[:], ACT.Exp)
                um = smallp.tile([128, S, G], bf16, tag="um")
                nc.vector.tensor_tensor(
                    um[:].rearrange("p s g -> p (s g)"),
                    u0[:],
                    maskall[:, sp * S : (sp + 1) * S, :].rearrange(
                        "p s g -> p (s g)"
                    ),
                    op=ALU.mult,
                )

                # --- weighted msg + trees over g (all 2x tt ops) ---
                rhs = smallp.tile([128, S, D + 1], bf16, tag="rhs")
                wm = workp.tile([128, S, D, G], bf16, tag="wm")
                umb = (
                    um[:]
                    .rearrange("p s (o g) -> p s o g", o=1)
                    .broadcast_to([128, S, D, G])
                )
                nc.vector.tensor_tensor(wm[:], mg[:], umb, op=ALU.mult)
                w4 = workp.tile([128, S, D, 4], bf16, tag="w4")
                nc.vector.tensor_tensor(
                    w4[:], wm[:, :, :, 0:4], wm[:, :, :, 4:8], op=ALU.add
                )
                w2 = workp.tile([128, S, D, 2], bf16, tag="w2")
                nc.vector.tensor_tensor(
                    w2[:], w4[:, :, :, 0:2], w4[:, :, :, 2:4], op=ALU.add
                )
                nc.vector.tensor_tensor(
                    rhs[:, :, 0:D].rearrange("p s (d o) -> p s d o", o=1),
                    w2[:, :, :, 0:1],
                    w2[:, :, :, 1:2],
                    op=ALU.add,
                )
                # S_n tree over g
                s4 = smallp.tile([128, S, 4], bf16, tag="s4")
                nc.vector.tensor_tensor(
                    s4[:], um[:, :, 0:4], um[:, :, 4:8], op=ALU.add
                )
                s2 = smallp.tile([128, S, 2], bf16, tag="s2")
                nc.vector.tensor_tensor(
                    s2[:], s4[:, :, 0:2], s4[:, :, 2:4], op=ALU.add
                )
                nc.vector.tensor_tensor(
                    rhs[:, :, D : D + 1],
                    s2[:, :, 0:1],
                    s2[:, :, 1:2],
                    op=ALU.add,
                )

                # --- per-tile one-hot segment reduce on PE ---
                ob = obp.tile([128, S, D + 1], f32, tag="ob")
                for k in range(S):
                    t = sp * S + k
                    oh = ohp.tile([128, 128], bf16, tag="oh")
                    nc.vector.tensor_scalar(
                        oh[:], iota_t[:], relall[:, t : t + 1], None,
                        op0=ALU.is_equal,
                    )
                    ps = pstp.tile([128, D + 1], f32)
                    nc.tensor.matmul(ps[:], oh[:], rhs[:, k, :], start=True, stop=True)
                    nc.scalar.copy(ob[:, k, :], ps[:])
                nc.sync.dma_start(out=out_d[sp], in_=ob[:])

    nc.compile()
    return nc


def kernel(msg, x_i, x_j, e_ij, W, b, index, num_nodes):
    global LAST_EXEC_NS
    msg = np.ascontiguousarray(np.asarray(msg, dtype=np.float32))
    x_i = np.ascontiguousarray(np.asarray(x_i, dtype=np.float32))
    x_j = np.ascontiguousarray(np.asarray(x_j, dtype=np.float32))
    e_ij = np.ascontiguousarray(np.asarray(e_ij, dtype=np.float32))
    W = np.asarray(W, dtype=np.float32)
    bval = float(np.asarray(b, dtype=np.float32).reshape(-1)[0])
    idx = np.asarray(index).astype(np.int64).reshape(-1)
    N = int(np.asarray(num_nodes).reshape(()))
    E = idx.shape[0]

    # ---- host prep (untimed): pad edges into G-slot groups per node ----
    if np.any(np.diff(idx) < 0):
        order = np.argsort(idx, kind="stable")
    else:
        order = np.arange(E, dtype=np.int64)
    idx_s = idx[order]

    deg = np.bincount(idx_s, minlength=N)
    ngrp = -(-deg // G)
    B = int(ngrp.sum())
    bc = -(-B // NCORES)
    bc = -(-bc // 1024) * 1024  # per-core groups, multiple of 128*S (super-tiles)
    btot = bc * NCORES
    ntiles = bc // 128

    node_of_group = np.repeat(np.arange(N, dtype=np.int64), ngrp)
    node_of_group = np.concatenate(
        [node_of_group, np.full(btot - B, N, dtype=np.int64)]
    )

    gstart = np.zeros(N + 1, dtype=np.int64)
    np.cumsum(ngrp, out=gstart[1:])
    seg_start = np.zeros(N + 1, dtype=np.int64)
    np.cumsum(deg, out=seg_start[1:])
    rank_in_node = np.arange(E, dtype=np.int64) - seg_start[idx_s]
    slot = gstart[idx_s] * G + rank_in_node  # slot of each sorted edge

    nslots = btot * G
    perm = np.full(nslots, -1, dtype=np.int64)
    perm[slot] = order
    mask_f = (perm >= 0).astype(np.float32)
    src_idx = np.where(perm >= 0, perm, 0)

    nsup = ntiles // S
    SG = S * G

    # --- score pack: [C, nsup, 96, S, G, 128] feature-major bf16 ---
    # slot-column c = t*G*128 + g*128 + p ; rows = xj(0:32)|eij(32:64)|xi(64:96)
    # per-core slot layout mirror: slot id = ((tile*128)+p)*G + g
    sc = np.empty((NCORES, nsup, 96, SG * 128), dtype=np_fp8)
    # build index array mapping (tile,p,g) -> src edge, then transpose views
    si = src_idx.reshape(NCORES, nsup, S, 128, G)
    for arr, row0 in ((x_j, 0), (e_ij, 32), (x_i, 64)):
        # arr[si] -> [C, nsup, S, 128, G, 32] ; want [C, nsup, 32, S, G, 128]
        v = arr[si].astype(np_fp8).transpose(0, 1, 5, 2, 4, 3)
        sc[:, :, row0 : row0 + 32] = v.reshape(NCORES, nsup, 32, SG * 128)

    # --- msg pack: [C, nsup, 128, S, D, G] bf16, pad slots zeroed ---
    mgv = (msg[src_idx] * mask_f[:, None]).astype(np_bf16)
    mg = np.ascontiguousarray(
        mgv.reshape(NCORES, nsup, S, 128, G, D).transpose(0, 1, 3, 2, 5, 4)
    ).reshape(NCORES, nsup, 128, S * D * G)

    mk = mask_f.astype(np_bf16).reshape(NCORES, ntiles, 128, G)
    mks = [np.ascontiguousarray(mk[c].transpose(1, 0, 2)) for c in range(NCORES)]

    # per-tile dense rank of node within tile (always < 128), plus row->node map
    nog = node_of_group.reshape(NCORES, ntiles, 128)
    newseg = np.ones((NCORES, ntiles, 128), dtype=np.int64)
    newseg[:, :, 1:] = (np.diff(nog, axis=2) != 0).astype(np.int64)
    rank = np.cumsum(newseg, axis=2) - 1  # [C, T, 128] in [0, 128)
    rels = [
        np.ascontiguousarray(rank[c].T.astype(np.float32)) for c in range(NCORES)
    ]
    nodemap = np.full((NCORES, ntiles, 128), N, dtype=np.int64)
    ci, ti, _ = np.meshgrid(
        np.arange(NCORES), np.arange(ntiles), np.arange(128), indexing="ij"
    )
    nodemap[ci, ti, rank] = nog

    # rows: xj*W1 + eij*W1 + xi*W2  -> [W1, W1, W2]
    wcat = np.concatenate([W[:D, 0], W[:D, 0], W[D:, 0]])
    wcat = np.ascontiguousarray(wcat.reshape(96, 1)).astype(np_bf16)

    in_maps = [
        {
            "sc": np.ascontiguousarray(sc[c]),
            "mg": np.ascontiguousarray(mg[c]),
            "mask": mks[c],
            "rel": rels[c],
            "wcat": wcat,
        }
        for c in range(NCORES)
    ]

    key = (ntiles, bval)
    if key not in _PROGRAM_CACHE:
        _PROGRAM_CACHE[key] = _build_program(ntiles, bval)
    nc = _PROGRAM_CACHE[key]

    res = run_bass_kernel_spmd(nc, in_maps, core_ids=list(range(NCORES)))
    LAST_EXEC_NS = res.exec_time_ns

    acc = np.zeros((N + 1, D + 1), dtype=np.float32)
    for c in range(NCORES):
        o = (
            np.asarray(res.results[c]["out"], dtype=np.float32)
            .reshape(nsup, 128, S, D + 1)
            .transpose(0, 2, 1, 3)
            .reshape(-1, D + 1)
        )
        np.add.at(acc, nodemap[c].reshape(-1), o)
    out = acc[:N, :D] / (acc[:N, D : D + 1] + 1e-16)
    return out.astype(np.float32)
